# revision 20
# baseline (speedup 1.0000x reference)
"""Trainium2 Bass kernel for CausalTensionGraphLayer.

Math (reference factorization, with the value path folded through merge):
  a   = x @ w1[:D] + b1                [T, H]   (H = D/2)
  c   = x @ w1[D:]                     [T, H]
  u   = x @ wvm + k2,  wvm = wv_w @ (m2/2),  k2 = wv_b @ (m2/2)
  hid_w  = silu(a[t] + c[t-w-1])                (c, u are 0/k2 for t-w-1 < 0)
  tau2_w = 1 + tanh((hid_w @ w2 + b2)/2)        (= 2*sigmoid(logit))
  y      = x @ m1 + sum_w tau2_w[t] * u[t-w-1]  (+ merge_b)
  out    = LayerNorm(y) * gamma + beta

Folding m2 into the value projection on the host removes the entire
msg @ m2 matmul block (1M MACs/token) and the m2 weight load; the whole
kernel then runs feature-major (tokens on the free axis), msg is added
into the y PSUM with a 1-step identity matmul, and LayerNorm statistics
are computed with ones-matmul partition reductions on the PE.

Sharding: data-parallel over the B*T = 8192 token rows, 1024 own tokens
per core plus a 4-row halo (zeros at batch boundaries).  No collectives.
The output leaves the device feature-major [D, NTOK] (bf16 values cast
to fp32 by the SWDGE store); the host transposes.

Schedule: A (a, then c) -> B rounds r: tau(r-1) | u slab r | hs/silu(r) |
msg(r-1) -> D per quarter (y chains + stats + LN + store).  Input DMA:
x + w1 on sync (HWDGE) at full bandwidth; remaining weights on the
gpsimd SWDGE queue, gated behind the first a-eviction.
"""

from contextlib import ExitStack

import numpy as np
import ml_dtypes

import concourse.bass as bass
import concourse.bacc as bacc
import concourse.tile as tile
from concourse import mybir
from concourse.bass_utils import run_bass_kernel_spmd

BF16 = ml_dtypes.bfloat16

B, T, D = 2, 4096, 1024
H = D // 2
W = 4
EPS = 1e-5
NCORES = 8
NTOK = (B * T) // NCORES          # 1024 own tokens per core
HALO = W                          # 4
GRID = NTOK + HALO                # 1028 (halo + own)
NQ = 4                            # token quarters per core
QT = NTOK // NQ                   # 256 own tokens per quarter
KD = D // 128                     # 8 K-chunks over D
MH = H // 128                     # 4 M-tiles over H
MD = D // 128                     # 8 M-tiles over D

FP32 = mybir.dt.float32
I32 = mybir.dt.int32
BF = mybir.dt.bfloat16
AF = mybir.ActivationFunctionType
ALU = mybir.AluOpType


def build_nc(flags):
    use_gamma_beta, use_merge_b, use_b1, use_k2, b2_half = flags
    nc = bacc.Bacc(None, target_bir_lowering=False)

    xT = nc.dram_tensor("xT", [D, GRID], BF, kind="ExternalInput")
    w1a = nc.dram_tensor("w1a", [D, H], BF, kind="ExternalInput")
    w1c = nc.dram_tensor("w1c", [D, H], BF, kind="ExternalInput")
    wvm = nc.dram_tensor("wvm", [D, D], BF, kind="ExternalInput")
    m1 = nc.dram_tensor("m1", [D, D], BF, kind="ExternalInput")
    w2rep = nc.dram_tensor("w2rep", [H, 128], BF, kind="ExternalInput")
    ident = nc.dram_tensor("ident", [128, 128], BF, kind="ExternalInput")
    if use_b1:
        b1r = nc.dram_tensor("b1r", [128, MH], FP32, kind="ExternalInput")
    if use_k2:
        k2r = nc.dram_tensor("k2r", [128, MD], FP32, kind="ExternalInput")
    if use_gamma_beta:
        gamr = nc.dram_tensor("gamr", [128, MD], FP32, kind="ExternalInput")
        betr = nc.dram_tensor("betr", [128, MD], FP32, kind="ExternalInput")
    if use_merge_b:
        mbr = nc.dram_tensor("mbr", [128, MD], FP32, kind="ExternalInput")
    y = nc.dram_tensor("y", [D, NTOK], BF, kind="ExternalOutput")

    with tile.TileContext(nc) as tc, ExitStack() as ctx:
        persist = ctx.enter_context(tc.tile_pool(name="persist", bufs=1))
        hspool = ctx.enter_context(tc.tile_pool(name="hspool", bufs=2))
        hsspool = ctx.enter_context(tc.tile_pool(name="hsspool", bufs=4))
        pwpool = ctx.enter_context(tc.tile_pool(name="pwpool", bufs=4))
        ypool = ctx.enter_context(tc.tile_pool(name="ypool", bufs=3))
        lnpool = ctx.enter_context(tc.tile_pool(name="lnpool", bufs=1))
        ps_gate = ctx.enter_context(tc.tile_pool(name="ps_gate", bufs=2, space="PSUM"))
        ps_log = ctx.enter_context(tc.tile_pool(name="ps_log", bufs=1, space="PSUM"))
        ps_y = ctx.enter_context(tc.tile_pool(name="ps_y", bufs=3, space="PSUM"))

        xT_sb = persist.tile([128, KD, GRID], BF, tag="xT")
        w1a_sb = persist.tile([128, KD, H], BF, tag="w1a")
        w1c_sb = persist.tile([128, KD, H], BF, tag="w1c")
        w2rep_sb = persist.tile([128, MH, 128], BF, tag="w2rep")
        wvm_sb = persist.tile([128, KD, D], BF, tag="wvm")
        m1_sb = persist.tile([128, KD, D], BF, tag="m1")
        ident_sb = persist.tile([128, 128], BF, tag="ident")
        ones_sb = persist.tile([128, 128], BF, tag="ones")
        nc.vector.memset(ones_sb, 1.0)
        xT_r = xT.rearrange("(n p) t -> p n t", p=128)
        w1a_r = w1a.rearrange("(n p) m -> p n m", p=128)
        w1c_r = w1c.rearrange("(n p) m -> p n m", p=128)
        w2_r = w2rep.rearrange("(n p) m -> p n m", p=128)
        wvm_r = wvm.rearrange("(n p) m -> p n m", p=128)
        m1_r = m1.rearrange("(n p) m -> p n m", p=128)
        y_r = y.rearrange("(n p) t -> p n t", p=128)

        # Sync queue: x + w1 in consumption order, full HBM bandwidth
        # (the SWDGE stream below is gated behind the first a-eviction).
        nc.sync.dma_start(out=xT_sb[:, :, 0:260], in_=xT_r[:, :, 0:260])
        nc.sync.dma_start(out=w1a_sb[:, :, 0:256], in_=w1a_r[:, :, 0:256])
        nc.sync.dma_start(out=w1a_sb[:, :, 256:H], in_=w1a_r[:, :, 256:H])
        nc.sync.dma_start(out=xT_sb[:, :, 260:516], in_=xT_r[:, :, 260:516])
        nc.sync.dma_start(out=xT_sb[:, :, 516:772], in_=xT_r[:, :, 516:772])
        nc.sync.dma_start(out=xT_sb[:, :, 772:GRID], in_=xT_r[:, :, 772:GRID])
        nc.sync.dma_start(out=w1c_sb[:, :, 0:256], in_=w1c_r[:, :, 0:256])
        nc.sync.dma_start(out=w1c_sb[:, :, 256:H], in_=w1c_r[:, :, 256:H])
        for mc in range(2):
            nc.sync.dma_start(
                out=wvm_sb[:, :, mc * 512:(mc + 1) * 512],
                in_=wvm_r[:, :, mc * 512:(mc + 1) * 512],
            )
        nc.sync.dma_start(out=w2rep_sb[:, :, :], in_=w2_r[:, :, :])
        nc.sync.dma_start(out=m1_sb[:, :, 0:512], in_=m1_r[:, :, 0:512])
        nc.sync.dma_start(out=m1_sb[:, :, 512:D], in_=m1_r[:, :, 512:D])
        nc.sync.dma_start(out=ident_sb, in_=ident[:, :])
        if use_b1:
            b1_sb = persist.tile([128, MH], FP32, tag="b1")
            nc.sync.dma_start(out=b1_sb, in_=b1r[:, :])
        if use_k2:
            k2_sb = persist.tile([128, MD], FP32, tag="k2")
            nc.sync.dma_start(out=k2_sb, in_=k2r[:, :])
        if use_gamma_beta:
            gam_sb = persist.tile([128, MD], FP32, tag="gam")
            nc.sync.dma_start(out=gam_sb, in_=gamr[:, :])
            bet_sb = persist.tile([128, MD], FP32, tag="bet")
            nc.sync.dma_start(out=bet_sb, in_=betr[:, :])
        if use_merge_b:
            mb_sb = persist.tile([128, MD], FP32, tag="mb")
            nc.sync.dma_start(out=mb_sb, in_=mbr[:, :])

        magic_sb = persist.tile([128, 1], I32, tag="magic")
        nc.vector.memset(magic_sb, 0x5F3759DF)
        one_i = persist.tile([128, 1], I32, tag="onei")
        nc.vector.memset(one_i, 1)

        # Full-grid gate tensors (bf16, feature-major: tokens on free axis).
        a_g = persist.tile([128, MH, NTOK], BF, tag="a_g")
        c_g = persist.tile([128, MH, GRID], BF, tag="c_g")
        u_g = persist.tile([128, MD, GRID], BF, tag="u_g")
        tauqs = [
            persist.tile([128, W, QT], BF, tag=f"tau{q}", name=f"tau{q}")
            for q in range(NQ)
        ]
        msgqs = [
            persist.tile([128, MD, QT], BF, tag=f"msg{q}", name=f"msg{q}")
            for q in range(NQ)
        ]

        def gate_chain(wsb, dst, src0, dst0, n, mp, bias_sb):
            """k-dense chain for 2 m-tiles of x@wsb over grid cols
            [src0, src0+n), evicted into dst[:, 2mp:2mp+2, dst0:dst0+n].
            The 512-wide psum rows keep each j-chain inside one PSUM bank
            (a matmul output cannot cross the 2KB bank boundary)."""
            ps = ps_gate.tile([128, 2, 512], FP32, tag="g")
            for j in range(2):
                m = 2 * mp + j
                for k in range(KD):
                    nc.tensor.matmul(
                        ps[:, j, 0:n],
                        wsb[:, k, m * 128:(m + 1) * 128],
                        xT_sb[:, k, src0:src0 + n],
                        start=(k == 0),
                        stop=(k == KD - 1),
                    )
            if bias_sb is None:
                nc.scalar.activation(
                    out=dst[:, 2 * mp:2 * mp + 2, dst0:dst0 + n],
                    in_=ps[:, :, 0:n], func=AF.Copy,
                )
            else:
                for j in range(2):
                    m = 2 * mp + j
                    nc.scalar.activation(
                        out=dst[:, m, dst0:dst0 + n],
                        in_=ps[:, j, 0:n], func=AF.Identity,
                        bias=bias_sb[:, m:m + 1], scale=1.0,
                    )

        # ---- phase A: a (own tokens) then c (shifted grid) ---------------
        b1s = b1_sb if use_b1 else None
        for s in range(NQ):
            for mp in range(MH // 2):
                gate_chain(w1a_sb, a_g, HALO + QT * s, QT * s, QT, mp, b1s)
        for s in range(NQ):
            for mp in range(MH // 2):
                gate_chain(w1c_sb, c_g, QT * s, QT * s, 260, mp, None)

        # hs_w[t] = a[t] + c[t-w-1]: token t-w-1 lives at grid col t+o with
        # o = HALO-1-w; a_g is own-token-indexed, c_g/u_g grid-indexed.
        hsss = [[None, None] for _ in range(NQ)]

        def c1_block(q):
            g0 = q * QT
            for p in range(W // 2):
                hs = hspool.tile([128, MH, 2, QT], BF, tag="hs")
                for wi in range(2):
                    w = 2 * p + wi
                    o = HALO - 1 - w
                    nc.vector.tensor_add(
                        hs[:, :, wi, :],
                        a_g[:, :, g0:g0 + QT],
                        c_g[:, :, g0 + o:g0 + o + QT],
                    )
                hss = hsspool.tile([128, MH, 2, QT], BF, tag="hss")
                nc.scalar.activation(out=hss, in_=hs, func=AF.Silu)
                hsss[q][p] = hss

        def tau_block(q):
            tq = tauqs[q]
            for p in range(W // 2):
                pl = ps_log.tile([128, 512], FP32, tag="logit")
                for k in range(MH):
                    nc.tensor.matmul(
                        pl,
                        w2rep_sb[:, k, :],
                        hsss[q][p][:, k, :, :],
                        start=(k == 0),
                        stop=(k == MH - 1),
                    )
                nc.scalar.activation(
                    out=tq[:, 2 * p:2 * p + 2, :],
                    in_=pl.rearrange("p (a b) -> p a b", a=2),
                    func=AF.Tanh, scale=0.5, bias=float(b2_half),
                )
            # tq <- tanh+1 = 2*tau (the 1/2 lives in wvm).
            nc.vector.tensor_scalar_add(tq, tq, 1.0)

        def bcast(ap2d, nmid):
            """[128, N] AP -> [128, (0-step nmid), N] broadcast AP."""
            return bass.AP(
                tensor=ap2d.tensor, offset=ap2d.offset,
                ap=[ap2d.ap[0], [0, nmid], ap2d.ap[1]],
            )

        def c3_block(q):
            g0 = q * QT
            tq = tauqs[q]
            pw = []
            for w in range(W):
                o = HALO - 1 - w
                pt = pwpool.tile([128, MD, QT], BF, tag="pw")
                nc.vector.tensor_mul(
                    pt, bcast(tq[:, w, :], MD), u_g[:, :, g0 + o:g0 + o + QT]
                )
                pw.append(pt)
                if w == 1:
                    nc.vector.tensor_add(pw[1], pw[0], pw[1])
            nc.vector.tensor_add(pw[3], pw[2], pw[3])
            nc.vector.tensor_add(msgqs[q], pw[1], pw[3])

        # ---- phase B rounds: tau(r-1) | u slab r | C1(r) | msg(r-1) ------
        k2s = k2_sb if use_k2 else None
        for r in range(NQ):
            if r > 0:
                tau_block(r - 1)
            for mp in range(MD // 2):
                gate_chain(wvm_sb, u_g, QT * r, QT * r, 260, mp, k2s)
            c1_block(r)
            if r > 0:
                c3_block(r - 1)
        tau_block(NQ - 1)
        c3_block(NQ - 1)

        # ---- phase D: y = m1^T x + msg (feature-major), LN, store --------
        # Pipelined per quarter: stats/LN/store of quarter q are emitted
        # after the y-chains of quarter q+1 so the PE never waits on them.
        y_sbs = [None] * NQ

        def d_chains(q):
            g0 = q * QT
            msgq = msgqs[q]
            y_sb = ypool.tile([128, MD, QT], BF, tag="y_sb")
            y_sbs[q] = y_sb
            for dp in range(MD // 2):
                yps = ps_y.tile([128, 2, QT], FP32, tag="y")
                for j in range(2):
                    dt = 2 * dp + j
                    for k in range(KD):
                        nc.tensor.matmul(
                            yps[:, j, :],
                            m1_sb[:, k, dt * 128:(dt + 1) * 128],
                            xT_sb[:, k, HALO + g0:HALO + g0 + QT],
                            start=(k == 0),
                            stop=False,
                        )
                    nc.tensor.matmul(
                        yps[:, j, :],
                        ident_sb,
                        msgq[:, dt, :],
                        start=False,
                        stop=True,
                    )
                if use_merge_b:
                    for j in range(2):
                        nc.vector.tensor_scalar_add(
                            yps[:, j, :], yps[:, j, :],
                            mb_sb[:, 2 * dp + j:2 * dp + j + 1],
                        )
                nc.scalar.activation(
                    out=y_sb[:, 2 * dp:2 * dp + 2, :], in_=yps, func=AF.Copy
                )

        def d_finish(q):
            g0 = q * QT
            y_sb = y_sbs[q]
            y2_sb = ypool.tile([128, MD, QT], BF, tag="y2_sb", bufs=2)
            nc.vector.tensor_mul(y2_sb, y_sb, y_sb)
            # Sum over all 1024 features: ones-matmul reduces partitions,
            # chaining over the 8 d-tiles accumulates the rest.  Results
            # land broadcast across partitions: [:, 0:256]=sum, [256:]=sumsq.
            st = ps_log.tile([128, 512], FP32, tag="logit")
            for dt in range(MD):
                nc.tensor.matmul(
                    st[:, 0:QT], ones_sb, y_sb[:, dt, :],
                    start=(dt == 0), stop=(dt == MD - 1),
                )
            for dt in range(MD):
                nc.tensor.matmul(
                    st[:, QT:2 * QT], ones_sb, y2_sb[:, dt, :],
                    start=(dt == 0), stop=(dt == MD - 1),
                )
            mean = lnpool.tile([128, QT], FP32, tag="mean")
            nc.vector.tensor_scalar_mul(mean, st[:, 0:QT], 1.0 / D)
            veps = lnpool.tile([128, QT], FP32, tag="veps")
            nc.vector.tensor_scalar(   # sumsq/D + eps
                out=veps, in0=st[:, QT:2 * QT], scalar1=1.0 / D, scalar2=EPS,
                op0=ALU.mult, op1=ALU.add,
            )
            m2e = lnpool.tile([128, QT], FP32, tag="m2e")
            nc.vector.scalar_tensor_tensor(   # mean^2
                out=m2e, in0=mean, scalar=1.0, in1=mean,
                op0=ALU.mult, op1=ALU.mult,
            )
            nc.vector.tensor_tensor(veps, veps, m2e, op=ALU.subtract)
            # rstd = rsqrt(veps) via bit-trick seed + 2 Newton steps.
            rbits = lnpool.tile([128, QT], I32, tag="rbits")
            nc.vector.tensor_scalar(
                out=rbits, in0=veps.bitcast(I32), scalar1=one_i[:, 0:1],
                scalar2=None, op0=ALU.arith_shift_right,
            )
            nc.vector.tensor_tensor(
                out=rbits, in0=magic_sb.to_broadcast([128, QT]), in1=rbits,
                op=ALU.subtract,
            )
            rstd = rbits.bitcast(FP32)
            for _ in range(1):
                nt1 = lnpool.tile([128, QT], FP32, tag="nt1")
                nc.vector.tensor_mul(nt1, rstd, rstd)
                nc.vector.tensor_mul(nt1, nt1, veps)
                nc.vector.tensor_scalar(
                    out=nt1, in0=nt1, scalar1=-0.5, scalar2=1.5,
                    op0=ALU.mult, op1=ALU.add,
                )
                nc.vector.tensor_mul(rstd, rstd, nt1)
            mean_bf = lnpool.tile([128, QT], BF, tag="mean_bf")
            nc.vector.tensor_copy(mean_bf, mean)
            rstd_bf = lnpool.tile([128, QT], BF, tag="rstd_bf")
            nc.vector.tensor_copy(rstd_bf, rstd)
            yout = ypool.tile([128, MD, QT], BF, tag="yout", bufs=2)
            nc.vector.tensor_tensor(
                yout, y_sb, bcast(mean_bf[:, :], MD), op=ALU.subtract
            )
            nc.vector.tensor_mul(yout, yout, bcast(rstd_bf[:, :], MD))
            if use_gamma_beta:
                for dt in range(MD):
                    nc.vector.tensor_scalar(
                        out=yout[:, dt, :], in0=yout[:, dt, :],
                        scalar1=gam_sb[:, dt:dt + 1],
                        scalar2=bet_sb[:, dt:dt + 1],
                        op0=ALU.mult, op1=ALU.add,
                    )
            nc.sync.dma_start(out=y_r[:, :, g0:g0 + QT], in_=yout)

        d_chains(0)
        for q in range(1, NQ):
            d_chains(q)
            d_finish(q - 1)
        d_finish(NQ - 1)
    nc.compile()
    return nc


_CACHE: dict = {}


def _get_nc(flags):
    if flags not in _CACHE:
        _CACHE[flags] = build_nc(flags)
    return _CACHE[flags]


def kernel(x, w1, b1, w2, b2, wv_w, wv_b, merge_w, merge_b, gamma, beta):
    x = np.asarray(x, dtype=np.float32)
    w1 = np.asarray(w1, dtype=np.float32)
    b1 = np.asarray(b1, dtype=np.float32)
    w2 = np.asarray(w2, dtype=np.float32)
    b2 = np.asarray(b2, dtype=np.float32)
    wv_w = np.asarray(wv_w, dtype=np.float32)
    wv_b = np.asarray(wv_b, dtype=np.float32)
    merge_w = np.asarray(merge_w, dtype=np.float32)
    merge_b = np.asarray(merge_b, dtype=np.float32)
    gamma = np.asarray(gamma, dtype=np.float32)
    beta = np.asarray(beta, dtype=np.float32)

    m2h = 0.5 * merge_w[D:]
    wvm = wv_w @ m2h
    k2 = wv_b @ m2h
    use_gamma_beta = not (np.all(gamma == 1.0) and np.all(beta == 0.0))
    use_merge_b = bool(np.any(merge_b != 0.0))
    use_b1 = bool(np.any(b1 != 0.0))
    use_k2 = bool(np.any(k2 != 0.0))
    b2_half = 0.5 * float(b2[0])
    flags = (use_gamma_beta, use_merge_b, use_b1, use_k2, b2_half)
    nc = _get_nc(flags)

    x2 = x.reshape(B * T, D)
    shared = {
        "w1a": w1[:D].astype(BF16),
        "w1c": w1[D:].astype(BF16),
        "wvm": wvm.astype(BF16),
        "m1": merge_w[:D].astype(BF16),
        "w2rep": np.ascontiguousarray(
            np.broadcast_to(w2.reshape(H, 1), (H, 128))
        ).astype(BF16),
        "ident": np.eye(128, dtype=np.float32).astype(BF16),
    }
    if use_b1:
        shared["b1r"] = np.ascontiguousarray(b1.reshape(MH, 128).T)
    if use_k2:
        shared["k2r"] = np.ascontiguousarray(k2.reshape(MD, 128).T)
    if use_gamma_beta:
        shared["gamr"] = np.ascontiguousarray(gamma.reshape(MD, 128).T)
        shared["betr"] = np.ascontiguousarray(beta.reshape(MD, 128).T)
    if use_merge_b:
        shared["mbr"] = np.ascontiguousarray(merge_b.reshape(MD, 128).T)

    in_maps = []
    for c in range(NCORES):
        t0 = c * NTOK
        xs = np.zeros((GRID, D), np.float32)
        xs[HALO:] = x2[t0:t0 + NTOK]
        if t0 % T != 0:  # halo stays inside the same batch element
            xs[:HALO] = x2[t0 - HALO:t0]
        m = dict(shared)
        m["xT"] = np.ascontiguousarray(xs.T).astype(BF16)
        in_maps.append(m)

    res = run_bass_kernel_spmd(nc, in_maps, core_ids=list(range(NCORES)))
    out = np.concatenate([r["y"].T for r in res.results], axis=0)
    return out.reshape(B, T, D).astype(np.float32)


# revision 22
# speedup vs baseline: 1.2615x; 1.2615x over previous
"""Trainium2 Bass kernel for CausalTensionGraphLayer.

Math (reference factorization, with the value path folded through merge):
  a   = x @ w1[:D] + b1                [T, H]   (H = D/2)
  c   = x @ w1[D:]                     [T, H]
  u   = x @ wvm + k2,  wvm = wv_w @ (m2/2),  k2 = wv_b @ (m2/2)
  hid_w  = silu(a[t] + c[t-w-1])                (c, u are 0/k2 for t-w-1 < 0)
  tau2_w = 1 + tanh((hid_w @ w2 + b2)/2)        (= 2*sigmoid(logit))
  y      = x @ m1 + sum_w tau2_w[t] * u[t-w-1]  (+ merge_b)
  out    = LayerNorm(y) * gamma + beta

Folding m2 into the value projection on the host removes the entire
msg @ m2 matmul block (1M MACs/token) and the m2 weight load; the whole
kernel then runs feature-major (tokens on the free axis), msg is added
into the y PSUM with a 1-step identity matmul, and LayerNorm statistics
are computed with ones-matmul partition reductions on the PE.

Sharding: data-parallel over the B*T = 8192 token rows, 1024 own tokens
per core plus a 4-row halo (zeros at batch boundaries).  No collectives.
The output leaves the device feature-major [D, NTOK] (bf16 values cast
to fp32 by the SWDGE store); the host transposes.

Schedule: A (a, then c) -> B rounds r: tau(r-1) | u slab r | hs/silu(r) |
msg(r-1) -> D per quarter (y chains + stats + LN + store).  Input DMA:
x + w1 on sync (HWDGE) at full bandwidth; remaining weights on the
gpsimd SWDGE queue, gated behind the first a-eviction.
"""

from contextlib import ExitStack

import numpy as np
import ml_dtypes

import concourse.bass as bass
import concourse.bacc as bacc
import concourse.tile as tile
from concourse import mybir
from concourse.bass_utils import run_bass_kernel_spmd

BF16 = ml_dtypes.bfloat16

B, T, D = 2, 4096, 1024
H = D // 2
W = 4
EPS = 1e-5
NCORES = 8
NTOK = (B * T) // NCORES          # 1024 own tokens per core
HALO = W                          # 4
GRID = NTOK + HALO                # 1028 (halo + own)
NQ = 4                            # token quarters per core
QT = NTOK // NQ                   # 256 own tokens per quarter
KD = D // 128                     # 8 K-chunks over D
MH = H // 128                     # 4 M-tiles over H
MD = D // 128                     # 8 M-tiles over D

FP32 = mybir.dt.float32
I32 = mybir.dt.int32
BF = mybir.dt.bfloat16
F8 = mybir.dt.float8e4
GRID8 = 1040  # fp8 x copy padded so the k-dim stride is a multiple of 16 (DoubleRow AP rule)
AF = mybir.ActivationFunctionType
ALU = mybir.AluOpType


def build_nc(flags):
    use_gamma_beta, use_merge_b, use_b1, use_k2, b2_half = flags
    nc = bacc.Bacc(None, target_bir_lowering=False)

    xT = nc.dram_tensor("xT", [D, GRID], BF, kind="ExternalInput")
    xT8 = nc.dram_tensor("xT8", [D, GRID], F8, kind="ExternalInput")
    w1a = nc.dram_tensor("w1a", [D, H], F8, kind="ExternalInput")
    w1c = nc.dram_tensor("w1c", [D, H], F8, kind="ExternalInput")
    wvm = nc.dram_tensor("wvm", [D, D], BF, kind="ExternalInput")
    m1 = nc.dram_tensor("m1", [D, D], BF, kind="ExternalInput")
    w2rep = nc.dram_tensor("w2rep", [H, 128], F8, kind="ExternalInput")
    ident = nc.dram_tensor("ident", [128, 128], BF, kind="ExternalInput")
    if use_b1:
        b1r = nc.dram_tensor("b1r", [128, MH], FP32, kind="ExternalInput")
    if use_k2:
        k2r = nc.dram_tensor("k2r", [128, MD], FP32, kind="ExternalInput")
    if use_gamma_beta:
        gamr = nc.dram_tensor("gamr", [128, MD], FP32, kind="ExternalInput")
        betr = nc.dram_tensor("betr", [128, MD], FP32, kind="ExternalInput")
    if use_merge_b:
        mbr = nc.dram_tensor("mbr", [128, MD], FP32, kind="ExternalInput")
    y = nc.dram_tensor("y", [D, NTOK], BF, kind="ExternalOutput")

    with tile.TileContext(nc) as tc, ExitStack() as ctx:
        persist = ctx.enter_context(tc.tile_pool(name="persist", bufs=1))
        hspool = ctx.enter_context(tc.tile_pool(name="hspool", bufs=2))
        hsspool = ctx.enter_context(tc.tile_pool(name="hsspool", bufs=4))
        pwpool = ctx.enter_context(tc.tile_pool(name="pwpool", bufs=4))
        ypool = ctx.enter_context(tc.tile_pool(name="ypool", bufs=3))
        lnpool = ctx.enter_context(tc.tile_pool(name="lnpool", bufs=1))
        ps_gate = ctx.enter_context(tc.tile_pool(name="ps_gate", bufs=2, space="PSUM"))
        ps_log = ctx.enter_context(tc.tile_pool(name="ps_log", bufs=1, space="PSUM"))
        ps_y = ctx.enter_context(tc.tile_pool(name="ps_y", bufs=3, space="PSUM"))

        xT_sb = persist.tile([128, KD, GRID], BF, tag="xT")
        xT8_sb = persist.tile([128, KD, GRID8], F8, tag="xT8")
        w1a_sb = persist.tile([128, KD, H], F8, tag="w1a")
        w1c_sb = persist.tile([128, KD, H], F8, tag="w1c")
        w2rep_sb = persist.tile([128, MH, 128], F8, tag="w2rep")
        wvm_sb = persist.tile([128, KD, D], BF, tag="wvm")
        m1_sb = persist.tile([128, KD, D], BF, tag="m1")
        ident_sb = persist.tile([128, 128], BF, tag="ident")
        ones_sb = persist.tile([128, 128], BF, tag="ones")
        nc.vector.memset(ones_sb, 1.0)
        xT_r = xT.rearrange("(n p) t -> p n t", p=128)
        xT8_r = xT8.rearrange("(n p) t -> p n t", p=128)
        w1a_r = w1a.rearrange("(n p) m -> p n m", p=128)
        w1c_r = w1c.rearrange("(n p) m -> p n m", p=128)
        w2_r = w2rep.rearrange("(n p) m -> p n m", p=128)
        wvm_r = wvm.rearrange("(n p) m -> p n m", p=128)
        m1_r = m1.rearrange("(n p) m -> p n m", p=128)
        y_r = y.rearrange("(n p) t -> p n t", p=128)

        # Sync queue: x + w1 in consumption order, full HBM bandwidth
        # (the SWDGE stream below is gated behind the first a-eviction).
        nc.sync.dma_start(out=xT8_sb[:, :, 0:516], in_=xT8_r[:, :, 0:516])
        nc.sync.dma_start(out=w1a_sb[:, :, :], in_=w1a_r[:, :, :])
        nc.sync.dma_start(out=xT8_sb[:, :, 516:GRID], in_=xT8_r[:, :, 516:GRID])
        nc.sync.dma_start(out=w1c_sb[:, :, :], in_=w1c_r[:, :, :])
        nc.sync.dma_start(out=xT_sb[:, :, 0:516], in_=xT_r[:, :, 0:516])
        nc.sync.dma_start(out=xT_sb[:, :, 516:GRID], in_=xT_r[:, :, 516:GRID])
        for mc in range(2):
            nc.sync.dma_start(
                out=wvm_sb[:, :, mc * 512:(mc + 1) * 512],
                in_=wvm_r[:, :, mc * 512:(mc + 1) * 512],
            )
        nc.sync.dma_start(out=w2rep_sb[:, :, :], in_=w2_r[:, :, :])
        nc.sync.dma_start(out=m1_sb[:, :, 0:512], in_=m1_r[:, :, 0:512])
        nc.sync.dma_start(out=m1_sb[:, :, 512:D], in_=m1_r[:, :, 512:D])
        nc.sync.dma_start(out=ident_sb, in_=ident[:, :])
        if use_b1:
            b1_sb = persist.tile([128, MH], FP32, tag="b1")
            nc.sync.dma_start(out=b1_sb, in_=b1r[:, :])
        if use_k2:
            k2_sb = persist.tile([128, MD], FP32, tag="k2")
            nc.sync.dma_start(out=k2_sb, in_=k2r[:, :])
        if use_gamma_beta:
            gam_sb = persist.tile([128, MD], FP32, tag="gam")
            nc.sync.dma_start(out=gam_sb, in_=gamr[:, :])
            bet_sb = persist.tile([128, MD], FP32, tag="bet")
            nc.sync.dma_start(out=bet_sb, in_=betr[:, :])
        if use_merge_b:
            mb_sb = persist.tile([128, MD], FP32, tag="mb")
            nc.sync.dma_start(out=mb_sb, in_=mbr[:, :])

        magic_sb = persist.tile([128, 1], I32, tag="magic")
        nc.vector.memset(magic_sb, 0x5F3759DF)
        one_i = persist.tile([128, 1], I32, tag="onei")
        nc.vector.memset(one_i, 1)

        # Full-grid gate tensors (bf16, feature-major: tokens on free axis).
        a_g = persist.tile([128, MH, NTOK], BF, tag="a_g")
        c_g = persist.tile([128, MH, GRID], BF, tag="c_g")
        u_g = persist.tile([128, MD, GRID], BF, tag="u_g")
        tauqs = [
            persist.tile([128, W, QT], BF, tag=f"tau{q}", name=f"tau{q}")
            for q in range(NQ)
        ]
        msgqs = [
            persist.tile([128, MD, QT], BF, tag=f"msg{q}", name=f"msg{q}")
            for q in range(NQ)
        ]

        DR = mybir.MatmulPerfMode.DoubleRow

        def gate_chain(wsb, dst, src0, dst0, n, m, bias_sb, fp8=False):
            """k-dense chain for one m-tile of x@wsb over grid cols
            [src0, src0+n), evicted into dst[:, m, dst0:dst0+n].  One PSUM
            bank per chain (a matmul output cannot cross banks).  fp8
            chains run double-pumped (2 fp8 weights per PE cell)."""
            ps = ps_gate.tile([128, 512], FP32, tag="g")
            if fp8:
                for k in range(0, KD, 2):
                    nc.tensor.matmul(
                        ps[:, 0:n],
                        wsb[:, k:k + 2, m * 128:(m + 1) * 128],
                        xT8_sb[:, k:k + 2, src0:src0 + n],
                        start=(k == 0),
                        stop=(k == KD - 2),
                        perf_mode=DR,
                    )
            else:
                for k in range(KD):
                    nc.tensor.matmul(
                        ps[:, 0:n],
                        wsb[:, k, m * 128:(m + 1) * 128],
                        xT_sb[:, k, src0:src0 + n],
                        start=(k == 0),
                        stop=(k == KD - 1),
                    )
            if bias_sb is None:
                nc.scalar.activation(
                    out=dst[:, m, dst0:dst0 + n], in_=ps[:, 0:n], func=AF.Copy
                )
            else:
                nc.scalar.activation(
                    out=dst[:, m, dst0:dst0 + n],
                    in_=ps[:, 0:n], func=AF.Identity,
                    bias=bias_sb[:, m:m + 1], scale=1.0,
                )

        # ---- phase A: a (own tokens) then c (shifted grid) ---------------
        b1s = b1_sb if use_b1 else None
        for s in range(2):
            for m in range(MH):
                gate_chain(w1a_sb, a_g, HALO + 512 * s, 512 * s, 512, m, b1s, True)
        for s in range(2):
            for m in range(MH):
                gate_chain(w1c_sb, c_g, 512 * s, 512 * s, 512, m, None, True)
        for m in range(MH):  # c tail cols [1024, 1028)
            gate_chain(w1c_sb, c_g, 1024, 1024, HALO, m, None, True)

        # hs_w[t] = a[t] + c[t-w-1]: token t-w-1 lives at grid col t+o with
        # o = HALO-1-w; a_g is own-token-indexed, c_g/u_g grid-indexed.
        hsss = [[None, None] for _ in range(NQ)]

        def c1_block(q):
            g0 = q * QT
            for p in range(W // 2):
                hs = hspool.tile([128, MH, 2, QT], BF, tag="hs")
                for wi in range(2):
                    w = 2 * p + wi
                    o = HALO - 1 - w
                    nc.vector.tensor_add(
                        hs[:, :, wi, :],
                        a_g[:, :, g0:g0 + QT],
                        c_g[:, :, g0 + o:g0 + o + QT],
                    )
                hss = hsspool.tile([128, MH, 2, QT], F8, tag="hss")
                nc.scalar.activation(out=hss, in_=hs, func=AF.Silu, scale=1.0 / 64.0)
                hsss[q][p] = hss

        def tau_block(q):
            tq = tauqs[q]
            for p in range(W // 2):
                pl = ps_log.tile([128, 512], FP32, tag="logit")
                for k in range(0, MH, 2):
                    nc.tensor.matmul(
                        pl,
                        w2rep_sb[:, k:k + 2, :],
                        hsss[q][p][:, k:k + 2, :, :],
                        start=(k == 0),
                        stop=(k == MH - 2),
                        perf_mode=DR,
                    )
                nc.scalar.activation(
                    out=tq[:, 2 * p:2 * p + 2, :],
                    in_=pl.rearrange("p (a b) -> p a b", a=2),
                    func=AF.Tanh, scale=0.5 / 64.0, bias=float(b2_half),
                )
            # tq <- tanh+1 = 2*tau (the 1/2 lives in wvm).
            nc.vector.tensor_scalar_add(tq, tq, 1.0)

        def bcast(ap2d, nmid):
            """[128, N] AP -> [128, (0-step nmid), N] broadcast AP."""
            return bass.AP(
                tensor=ap2d.tensor, offset=ap2d.offset,
                ap=[ap2d.ap[0], [0, nmid], ap2d.ap[1]],
            )

        def c3_block(q):
            g0 = q * QT
            tq = tauqs[q]
            pw = []
            for w in range(W):
                o = HALO - 1 - w
                pt = pwpool.tile([128, MD, QT], BF, tag="pw")
                nc.vector.tensor_mul(
                    pt, bcast(tq[:, w, :], MD), u_g[:, :, g0 + o:g0 + o + QT]
                )
                pw.append(pt)
                if w == 1:
                    nc.vector.tensor_add(pw[1], pw[0], pw[1])
            nc.vector.tensor_add(pw[3], pw[2], pw[3])
            nc.vector.tensor_add(msgqs[q], pw[1], pw[3])

        # ---- phase B rounds: tau(r-1) | u slab r | C1(r) | msg(r-1) ------
        k2s = k2_sb if use_k2 else None
        for r in range(NQ):
            if r > 0:
                tau_block(r - 1)
            for m in range(MD):
                gate_chain(wvm_sb, u_g, QT * r, QT * r, 260, m, k2s)
            c1_block(r)
            if r > 0:
                c3_block(r - 1)
        tau_block(NQ - 1)
        c3_block(NQ - 1)

        # ---- phase D: y = m1^T x + msg (feature-major), LN, store --------
        # Pipelined per quarter: stats/LN/store of quarter q are emitted
        # after the y-chains of quarter q+1 so the PE never waits on them.
        y_sbs = [None] * NQ

        def d_chains(q):
            g0 = q * QT
            msgq = msgqs[q]
            y_sb = ypool.tile([128, MD, QT], BF, tag="y_sb")
            y_sbs[q] = y_sb
            for dp in range(MD // 2):
                yps = ps_y.tile([128, 2, QT], FP32, tag="y")
                for j in range(2):
                    dt = 2 * dp + j
                    for k in range(KD):
                        nc.tensor.matmul(
                            yps[:, j, :],
                            m1_sb[:, k, dt * 128:(dt + 1) * 128],
                            xT_sb[:, k, HALO + g0:HALO + g0 + QT],
                            start=(k == 0),
                            stop=False,
                        )
                    nc.tensor.matmul(
                        yps[:, j, :],
                        ident_sb,
                        msgq[:, dt, :],
                        start=False,
                        stop=True,
                    )
                if use_merge_b:
                    for j in range(2):
                        nc.vector.tensor_scalar_add(
                            yps[:, j, :], yps[:, j, :],
                            mb_sb[:, 2 * dp + j:2 * dp + j + 1],
                        )
                nc.scalar.activation(
                    out=y_sb[:, 2 * dp:2 * dp + 2, :], in_=yps, func=AF.Copy
                )

        def d_finish(q):
            g0 = q * QT
            y_sb = y_sbs[q]
            y2_sb = ypool.tile([128, MD, QT], BF, tag="y2_sb", bufs=2)
            nc.vector.tensor_mul(y2_sb, y_sb, y_sb)
            # Sum over all 1024 features: ones-matmul reduces partitions,
            # chaining over the 8 d-tiles accumulates the rest.  Results
            # land broadcast across partitions: [:, 0:256]=sum, [256:]=sumsq.
            st = ps_log.tile([128, 512], FP32, tag="logit")
            for dt in range(MD):
                nc.tensor.matmul(
                    st[:, 0:QT], ones_sb, y_sb[:, dt, :],
                    start=(dt == 0), stop=(dt == MD - 1),
                )
            for dt in range(MD):
                nc.tensor.matmul(
                    st[:, QT:2 * QT], ones_sb, y2_sb[:, dt, :],
                    start=(dt == 0), stop=(dt == MD - 1),
                )
            mean = lnpool.tile([128, QT], FP32, tag="mean")
            nc.vector.tensor_scalar_mul(mean, st[:, 0:QT], 1.0 / D)
            veps = lnpool.tile([128, QT], FP32, tag="veps")
            nc.vector.tensor_scalar(   # sumsq/D + eps
                out=veps, in0=st[:, QT:2 * QT], scalar1=1.0 / D, scalar2=EPS,
                op0=ALU.mult, op1=ALU.add,
            )
            m2e = lnpool.tile([128, QT], FP32, tag="m2e")
            nc.vector.scalar_tensor_tensor(   # mean^2
                out=m2e, in0=mean, scalar=1.0, in1=mean,
                op0=ALU.mult, op1=ALU.mult,
            )
            nc.vector.tensor_tensor(veps, veps, m2e, op=ALU.subtract)
            # rstd = rsqrt(veps) via bit-trick seed + 2 Newton steps.
            rbits = lnpool.tile([128, QT], I32, tag="rbits")
            nc.vector.tensor_scalar(
                out=rbits, in0=veps.bitcast(I32), scalar1=one_i[:, 0:1],
                scalar2=None, op0=ALU.arith_shift_right,
            )
            nc.vector.tensor_tensor(
                out=rbits, in0=magic_sb.to_broadcast([128, QT]), in1=rbits,
                op=ALU.subtract,
            )
            rstd = rbits.bitcast(FP32)
            for _ in range(1):
                nt1 = lnpool.tile([128, QT], FP32, tag="nt1")
                nc.vector.tensor_mul(nt1, rstd, rstd)
                nc.vector.tensor_mul(nt1, nt1, veps)
                nc.vector.tensor_scalar(
                    out=nt1, in0=nt1, scalar1=-0.5, scalar2=1.5,
                    op0=ALU.mult, op1=ALU.add,
                )
                nc.vector.tensor_mul(rstd, rstd, nt1)
            mean_bf = lnpool.tile([128, QT], BF, tag="mean_bf")
            nc.vector.tensor_copy(mean_bf, mean)
            rstd_bf = lnpool.tile([128, QT], BF, tag="rstd_bf")
            nc.vector.tensor_copy(rstd_bf, rstd)
            yout = ypool.tile([128, MD, QT], BF, tag="yout", bufs=2)
            nc.vector.tensor_tensor(
                yout, y_sb, bcast(mean_bf[:, :], MD), op=ALU.subtract
            )
            nc.vector.tensor_mul(yout, yout, bcast(rstd_bf[:, :], MD))
            if use_gamma_beta:
                for dt in range(MD):
                    nc.vector.tensor_scalar(
                        out=yout[:, dt, :], in0=yout[:, dt, :],
                        scalar1=gam_sb[:, dt:dt + 1],
                        scalar2=bet_sb[:, dt:dt + 1],
                        op0=ALU.mult, op1=ALU.add,
                    )
            nc.sync.dma_start(out=y_r[:, :, g0:g0 + QT], in_=yout)

        d_chains(0)
        for q in range(1, NQ):
            d_chains(q)
            d_finish(q - 1)
        d_finish(NQ - 1)
    nc.compile()
    return nc


_CACHE: dict = {}


def _get_nc(flags):
    if flags not in _CACHE:
        _CACHE[flags] = build_nc(flags)
    return _CACHE[flags]


def kernel(x, w1, b1, w2, b2, wv_w, wv_b, merge_w, merge_b, gamma, beta):
    x = np.asarray(x, dtype=np.float32)
    w1 = np.asarray(w1, dtype=np.float32)
    b1 = np.asarray(b1, dtype=np.float32)
    w2 = np.asarray(w2, dtype=np.float32)
    b2 = np.asarray(b2, dtype=np.float32)
    wv_w = np.asarray(wv_w, dtype=np.float32)
    wv_b = np.asarray(wv_b, dtype=np.float32)
    merge_w = np.asarray(merge_w, dtype=np.float32)
    merge_b = np.asarray(merge_b, dtype=np.float32)
    gamma = np.asarray(gamma, dtype=np.float32)
    beta = np.asarray(beta, dtype=np.float32)

    m2h = 0.5 * merge_w[D:]
    wvm = wv_w @ m2h
    k2 = wv_b @ m2h
    use_gamma_beta = not (np.all(gamma == 1.0) and np.all(beta == 0.0))
    use_merge_b = bool(np.any(merge_b != 0.0))
    use_b1 = bool(np.any(b1 != 0.0))
    use_k2 = bool(np.any(k2 != 0.0))
    b2_half = 0.5 * float(b2[0])
    flags = (use_gamma_beta, use_merge_b, use_b1, use_k2, b2_half)
    nc = _get_nc(flags)

    F8NP = ml_dtypes.float8_e4m3fn
    x2 = x.reshape(B * T, D)
    shared = {
        "w1a": (64.0 * w1[:D]).astype(F8NP),
        "w1c": (64.0 * w1[D:]).astype(F8NP),
        "wvm": wvm.astype(BF16),
        "m1": merge_w[:D].astype(BF16),
        "w2rep": np.ascontiguousarray(
            np.broadcast_to((64.0 * w2).reshape(H, 1), (H, 128))
        ).astype(F8NP),
        "ident": np.eye(128, dtype=np.float32).astype(BF16),
    }
    if use_b1:
        shared["b1r"] = np.ascontiguousarray(64.0 * b1.reshape(MH, 128).T)
    if use_k2:
        shared["k2r"] = np.ascontiguousarray(k2.reshape(MD, 128).T)
    if use_gamma_beta:
        shared["gamr"] = np.ascontiguousarray(gamma.reshape(MD, 128).T)
        shared["betr"] = np.ascontiguousarray(beta.reshape(MD, 128).T)
    if use_merge_b:
        shared["mbr"] = np.ascontiguousarray(merge_b.reshape(MD, 128).T)

    in_maps = []
    for c in range(NCORES):
        t0 = c * NTOK
        xs = np.zeros((GRID, D), np.float32)
        xs[HALO:] = x2[t0:t0 + NTOK]
        if t0 % T != 0:  # halo stays inside the same batch element
            xs[:HALO] = x2[t0 - HALO:t0]
        m = dict(shared)
        xsT = np.ascontiguousarray(xs.T)
        m["xT"] = xsT.astype(BF16)
        m["xT8"] = xsT.astype(F8NP)
        in_maps.append(m)

    res = run_bass_kernel_spmd(nc, in_maps, core_ids=list(range(NCORES)))
    out = np.concatenate([r["y"].T for r in res.results], axis=0)
    return out.reshape(B, T, D).astype(np.float32)


# revision 25
# speedup vs baseline: 1.2874x; 1.0205x over previous
"""Trainium2 Bass kernel for CausalTensionGraphLayer.

Math (reference factorization, with the value path folded through merge):
  a   = x @ w1[:D] + b1                [T, H]   (H = D/2)
  c   = x @ w1[D:]                     [T, H]
  u   = x @ wvm + k2,  wvm = wv_w @ (m2/2),  k2 = wv_b @ (m2/2)
  hid_w  = silu(a[t] + c[t-w-1])                (c, u are 0/k2 for t-w-1 < 0)
  tau2_w = 1 + tanh((hid_w @ w2 + b2)/2)        (= 2*sigmoid(logit))
  y      = x @ m1 + sum_w tau2_w[t] * u[t-w-1]  (+ merge_b)
  out    = LayerNorm(y) * gamma + beta

Folding m2 into the value projection on the host removes the entire
msg @ m2 matmul block (1M MACs/token) and the m2 weight load; the whole
kernel then runs feature-major (tokens on the free axis), msg is added
into the y PSUM with a 1-step identity matmul, and LayerNorm statistics
are computed with ones-matmul partition reductions on the PE.

Sharding: data-parallel over the B*T = 8192 token rows, 1024 own tokens
per core plus a 4-row halo (zeros at batch boundaries).  No collectives.
The output leaves the device feature-major [D, NTOK] (bf16 values cast
to fp32 by the SWDGE store); the host transposes.

Schedule: A (a, then c) -> B rounds r: tau(r-1) | u slab r | hs/silu(r) |
msg(r-1) -> D per quarter (y chains + stats + LN + store).  Input DMA:
x + w1 on sync (HWDGE) at full bandwidth; remaining weights on the
gpsimd SWDGE queue, gated behind the first a-eviction.
"""

from contextlib import ExitStack

import numpy as np
import ml_dtypes

import concourse.bass as bass
import concourse.bacc as bacc
import concourse.tile as tile
from concourse import mybir
from concourse.bass_utils import run_bass_kernel_spmd

BF16 = ml_dtypes.bfloat16

B, T, D = 2, 4096, 1024
H = D // 2
W = 4
EPS = 1e-5
NCORES = 8
NTOK = (B * T) // NCORES          # 1024 own tokens per core
HALO = W                          # 4
GRID = NTOK + HALO                # 1028 (halo + own)
NQ = 4                            # token quarters per core
QT = NTOK // NQ                   # 256 own tokens per quarter
KD = D // 128                     # 8 K-chunks over D
MH = H // 128                     # 4 M-tiles over H
MD = D // 128                     # 8 M-tiles over D

FP32 = mybir.dt.float32
I32 = mybir.dt.int32
BF = mybir.dt.bfloat16
F8 = mybir.dt.float8e4
GRID8 = 1040  # fp8 x copy padded so the k-dim stride is a multiple of 16 (DoubleRow AP rule)
AF = mybir.ActivationFunctionType
ALU = mybir.AluOpType


def build_nc(flags):
    use_gamma_beta, use_merge_b, use_b1, use_k2, b2_half = flags
    nc = bacc.Bacc(None, target_bir_lowering=False)

    xT = nc.dram_tensor("xT", [D, GRID], BF, kind="ExternalInput")
    xT8 = nc.dram_tensor("xT8", [D, GRID], F8, kind="ExternalInput")
    w1a = nc.dram_tensor("w1a", [D, H], F8, kind="ExternalInput")
    w1c = nc.dram_tensor("w1c", [D, H], F8, kind="ExternalInput")
    wvm = nc.dram_tensor("wvm", [D, D], BF, kind="ExternalInput")
    m1 = nc.dram_tensor("m1", [D, D], BF, kind="ExternalInput")
    w2rep = nc.dram_tensor("w2rep", [H, 128], F8, kind="ExternalInput")
    ident = nc.dram_tensor("ident", [128, 128], BF, kind="ExternalInput")
    if use_b1:
        b1r = nc.dram_tensor("b1r", [128, MH], FP32, kind="ExternalInput")
    if use_k2:
        k2r = nc.dram_tensor("k2r", [128, MD], FP32, kind="ExternalInput")
    if use_gamma_beta:
        gamr = nc.dram_tensor("gamr", [128, MD], FP32, kind="ExternalInput")
        betr = nc.dram_tensor("betr", [128, MD], FP32, kind="ExternalInput")
    if use_merge_b:
        mbr = nc.dram_tensor("mbr", [128, MD], FP32, kind="ExternalInput")
    y = nc.dram_tensor("y", [D, NTOK], BF, kind="ExternalOutput")

    with tile.TileContext(nc) as tc, ExitStack() as ctx:
        persist = ctx.enter_context(tc.tile_pool(name="persist", bufs=1))
        hspool = ctx.enter_context(tc.tile_pool(name="hspool", bufs=2))
        hsspool = ctx.enter_context(tc.tile_pool(name="hsspool", bufs=4))
        pwpool = ctx.enter_context(tc.tile_pool(name="pwpool", bufs=4))
        ypool = ctx.enter_context(tc.tile_pool(name="ypool", bufs=3))
        lnpool = ctx.enter_context(tc.tile_pool(name="lnpool", bufs=1))
        ps_gate = ctx.enter_context(tc.tile_pool(name="ps_gate", bufs=2, space="PSUM"))
        ps_log = ctx.enter_context(tc.tile_pool(name="ps_log", bufs=1, space="PSUM"))
        ps_y = ctx.enter_context(tc.tile_pool(name="ps_y", bufs=3, space="PSUM"))

        xT_sb = persist.tile([128, KD, GRID], BF, tag="xT")
        xT8_sb = persist.tile([128, KD, GRID8], F8, tag="xT8")
        w1a_sb = persist.tile([128, KD, H], F8, tag="w1a")
        w1c_sb = persist.tile([128, KD, H], F8, tag="w1c")
        w2rep_sb = persist.tile([128, MH, 128], F8, tag="w2rep")
        wvm_sb = persist.tile([128, KD, D], BF, tag="wvm")
        m1_sb = persist.tile([128, KD, D], BF, tag="m1")
        ident_sb = persist.tile([128, 128], BF, tag="ident")
        ones_sb = persist.tile([128, 128], BF, tag="ones")
        nc.vector.memset(ones_sb, 1.0)
        xT_r = xT.rearrange("(n p) t -> p n t", p=128)
        xT8_r = xT8.rearrange("(n p) t -> p n t", p=128)
        w1a_r = w1a.rearrange("(n p) m -> p n m", p=128)
        w1c_r = w1c.rearrange("(n p) m -> p n m", p=128)
        w2_r = w2rep.rearrange("(n p) m -> p n m", p=128)
        wvm_r = wvm.rearrange("(n p) m -> p n m", p=128)
        m1_r = m1.rearrange("(n p) m -> p n m", p=128)
        y_r = y.rearrange("(n p) t -> p n t", p=128)

        # Sync queue: x + w1 in consumption order, full HBM bandwidth
        # (the SWDGE stream below is gated behind the first a-eviction).
        nc.sync.dma_start(out=xT8_sb[:, :, 0:516], in_=xT8_r[:, :, 0:516])
        nc.sync.dma_start(out=w1a_sb[:, :, :], in_=w1a_r[:, :, :])
        nc.sync.dma_start(out=xT8_sb[:, :, 516:GRID], in_=xT8_r[:, :, 516:GRID])
        nc.sync.dma_start(out=w1c_sb[:, :, :], in_=w1c_r[:, :, :])
        nc.sync.dma_start(out=xT_sb[:, :, 0:516], in_=xT_r[:, :, 0:516])
        for mc in range(2):
            nc.sync.dma_start(
                out=wvm_sb[:, :, mc * 512:(mc + 1) * 512],
                in_=wvm_r[:, :, mc * 512:(mc + 1) * 512],
            )
        nc.sync.dma_start(out=xT_sb[:, :, 516:GRID], in_=xT_r[:, :, 516:GRID])
        nc.sync.dma_start(out=w2rep_sb[:, :, :], in_=w2_r[:, :, :])
        nc.sync.dma_start(out=m1_sb[:, :, 0:512], in_=m1_r[:, :, 0:512])
        nc.sync.dma_start(out=m1_sb[:, :, 512:D], in_=m1_r[:, :, 512:D])
        nc.sync.dma_start(out=ident_sb, in_=ident[:, :])
        if use_b1:
            b1_sb = persist.tile([128, MH], FP32, tag="b1")
            nc.sync.dma_start(out=b1_sb, in_=b1r[:, :])
        if use_k2:
            k2_sb = persist.tile([128, MD], FP32, tag="k2")
            nc.sync.dma_start(out=k2_sb, in_=k2r[:, :])
        if use_gamma_beta:
            gam_sb = persist.tile([128, MD], FP32, tag="gam")
            nc.sync.dma_start(out=gam_sb, in_=gamr[:, :])
            bet_sb = persist.tile([128, MD], FP32, tag="bet")
            nc.sync.dma_start(out=bet_sb, in_=betr[:, :])
        if use_merge_b:
            mb_sb = persist.tile([128, MD], FP32, tag="mb")
            nc.sync.dma_start(out=mb_sb, in_=mbr[:, :])

        magic_sb = persist.tile([128, 1], I32, tag="magic")
        nc.vector.memset(magic_sb, 0x5F3759DF)
        one_i = persist.tile([128, 1], I32, tag="onei")
        nc.vector.memset(one_i, 1)

        # Full-grid gate tensors (bf16, feature-major: tokens on free axis).
        a_g = persist.tile([128, MH, NTOK], BF, tag="a_g")
        c_g = persist.tile([128, MH, GRID], BF, tag="c_g")
        u_g = persist.tile([128, MD, GRID], BF, tag="u_g")
        tauqs = [
            persist.tile([128, W, QT], BF, tag=f"tau{q}", name=f"tau{q}")
            for q in range(NQ)
        ]
        msgqs = [
            persist.tile([128, MD, QT], BF, tag=f"msg{q}", name=f"msg{q}")
            for q in range(NQ)
        ]

        DR = mybir.MatmulPerfMode.DoubleRow

        def gate_chain(wsb, dst, src0, dst0, n, m, bias_sb, fp8=False):
            """k-dense chain for one m-tile of x@wsb over grid cols
            [src0, src0+n), evicted into dst[:, m, dst0:dst0+n].  One PSUM
            bank per chain (a matmul output cannot cross banks).  fp8
            chains run double-pumped (2 fp8 weights per PE cell)."""
            ps = ps_gate.tile([128, 512], FP32, tag="g")
            if fp8:
                for k in range(0, KD, 2):
                    nc.tensor.matmul(
                        ps[:, 0:n],
                        wsb[:, k:k + 2, m * 128:(m + 1) * 128],
                        xT8_sb[:, k:k + 2, src0:src0 + n],
                        start=(k == 0),
                        stop=(k == KD - 2),
                        perf_mode=DR,
                    )
            else:
                for k in range(KD):
                    nc.tensor.matmul(
                        ps[:, 0:n],
                        wsb[:, k, m * 128:(m + 1) * 128],
                        xT_sb[:, k, src0:src0 + n],
                        start=(k == 0),
                        stop=(k == KD - 1),
                    )
            if bias_sb is None:
                nc.scalar.activation(
                    out=dst[:, m, dst0:dst0 + n], in_=ps[:, 0:n], func=AF.Copy
                )
            else:
                nc.scalar.activation(
                    out=dst[:, m, dst0:dst0 + n],
                    in_=ps[:, 0:n], func=AF.Identity,
                    bias=bias_sb[:, m:m + 1], scale=1.0,
                )

        # ---- phase A: a (own tokens) then c (shifted grid) ---------------
        b1s = b1_sb if use_b1 else None
        for s in range(2):
            for m in range(MH):
                gate_chain(w1a_sb, a_g, HALO + 512 * s, 512 * s, 512, m, b1s, True)
        for s in range(2):
            for m in range(MH):
                gate_chain(w1c_sb, c_g, 512 * s, 512 * s, 512, m, None, True)
        for m in range(MH):  # c tail cols [1024, 1028)
            gate_chain(w1c_sb, c_g, 1024, 1024, HALO, m, None, True)

        # hs_w[t] = a[t] + c[t-w-1]: token t-w-1 lives at grid col t+o with
        # o = HALO-1-w; a_g is own-token-indexed, c_g/u_g grid-indexed.
        hsss = [[None, None] for _ in range(NQ)]

        def c1_block(q):
            g0 = q * QT
            for p in range(W // 2):
                hs = hspool.tile([128, MH, 2, QT], BF, tag="hs")
                for wi in range(2):
                    w = 2 * p + wi
                    o = HALO - 1 - w
                    nc.vector.tensor_add(
                        hs[:, :, wi, :],
                        a_g[:, :, g0:g0 + QT],
                        c_g[:, :, g0 + o:g0 + o + QT],
                    )
                hss = hsspool.tile([128, MH, 2, QT], F8, tag="hss")
                nc.scalar.activation(out=hss, in_=hs, func=AF.Silu, scale=1.0 / 64.0)
                hsss[q][p] = hss

        def tau_block(q):
            tq = tauqs[q]
            for p in range(W // 2):
                pl = ps_log.tile([128, 512], FP32, tag="logit")
                for k in range(0, MH, 2):
                    nc.tensor.matmul(
                        pl,
                        w2rep_sb[:, k:k + 2, :],
                        hsss[q][p][:, k:k + 2, :, :],
                        start=(k == 0),
                        stop=(k == MH - 2),
                        perf_mode=DR,
                    )
                nc.scalar.activation(
                    out=tq[:, 2 * p:2 * p + 2, :],
                    in_=pl.rearrange("p (a b) -> p a b", a=2),
                    func=AF.Tanh, scale=0.5 / 64.0, bias=float(b2_half),
                )
            # tq <- tanh+1 = 2*tau (the 1/2 lives in wvm).
            nc.vector.tensor_scalar_add(tq, tq, 1.0)

        def bcast(ap2d, nmid):
            """[128, N] AP -> [128, (0-step nmid), N] broadcast AP."""
            return bass.AP(
                tensor=ap2d.tensor, offset=ap2d.offset,
                ap=[ap2d.ap[0], [0, nmid], ap2d.ap[1]],
            )

        def c3_block(q):
            g0 = q * QT
            tq = tauqs[q]
            pw = []
            for w in range(W):
                o = HALO - 1 - w
                pt = pwpool.tile([128, MD, QT], BF, tag="pw")
                nc.vector.tensor_mul(
                    pt, bcast(tq[:, w, :], MD), u_g[:, :, g0 + o:g0 + o + QT]
                )
                pw.append(pt)
                if w == 1:
                    nc.vector.tensor_add(pw[1], pw[0], pw[1])
            nc.vector.tensor_add(pw[3], pw[2], pw[3])
            nc.vector.tensor_add(msgqs[q], pw[1], pw[3])

        # ---- phase B rounds: tau(r-1) | u slab r | C1(r) | msg(r-1) ------
        k2s = k2_sb if use_k2 else None
        for r in range(NQ):
            if r > 0:
                tau_block(r - 1)
            for m in range(MD):
                gate_chain(wvm_sb, u_g, QT * r, QT * r, 260, m, k2s)
            c1_block(r)
            if r > 0:
                c3_block(r - 1)
        tau_block(NQ - 1)
        c3_block(NQ - 1)

        # ---- phase D: y = m1^T x + msg (feature-major), LN, store --------
        # Pipelined per quarter: stats/LN/store of quarter q are emitted
        # after the y-chains of quarter q+1 so the PE never waits on them.
        y_sbs = [None] * NQ
        y2_sbs = [None] * NQ

        def d_chains(q):
            g0 = q * QT
            msgq = msgqs[q]
            y_sb = ypool.tile([128, MD, QT], BF, tag="y_sb")
            y_sbs[q] = y_sb
            y2_sbs[q] = ypool.tile([128, MD, QT], BF, tag="y2_sb", bufs=2, name="y2_sb")
            for dp in range(MD // 2):
                yps = ps_y.tile([128, 2, QT], FP32, tag="y")
                for j in range(2):
                    dt = 2 * dp + j
                    for k in range(KD):
                        nc.tensor.matmul(
                            yps[:, j, :],
                            m1_sb[:, k, dt * 128:(dt + 1) * 128],
                            xT_sb[:, k, HALO + g0:HALO + g0 + QT],
                            start=(k == 0),
                            stop=False,
                        )
                    nc.tensor.matmul(
                        yps[:, j, :],
                        ident_sb,
                        msgq[:, dt, :],
                        start=False,
                        stop=True,
                    )
                if use_merge_b:
                    for j in range(2):
                        nc.vector.tensor_scalar_add(
                            yps[:, j, :], yps[:, j, :],
                            mb_sb[:, 2 * dp + j:2 * dp + j + 1],
                        )
                nc.scalar.activation(
                    out=y_sb[:, 2 * dp:2 * dp + 2, :], in_=yps, func=AF.Copy
                )
                nc.scalar.activation(
                    out=y2_sbs[q][:, 2 * dp:2 * dp + 2, :], in_=yps,
                    func=AF.Square,
                )

        def d_finish(q):
            g0 = q * QT
            y_sb = y_sbs[q]
            y2_sb = y2_sbs[q]
            # Sum over all 1024 features: ones-matmul reduces partitions,
            # chaining over the 8 d-tiles accumulates the rest.  Results
            # land broadcast across partitions: [:, 0:256]=sum, [256:]=sumsq.
            st = ps_log.tile([128, 512], FP32, tag="logit")
            for dt in range(MD):
                nc.tensor.matmul(
                    st[:, 0:QT], ones_sb, y_sb[:, dt, :],
                    start=(dt == 0), stop=(dt == MD - 1),
                )
            for dt in range(MD):
                nc.tensor.matmul(
                    st[:, QT:2 * QT], ones_sb, y2_sb[:, dt, :],
                    start=(dt == 0), stop=(dt == MD - 1),
                )
            mean = lnpool.tile([128, QT], FP32, tag="mean")
            nc.vector.tensor_scalar_mul(mean, st[:, 0:QT], 1.0 / D)
            veps = lnpool.tile([128, QT], FP32, tag="veps")
            nc.vector.tensor_scalar(   # sumsq/D + eps
                out=veps, in0=st[:, QT:2 * QT], scalar1=1.0 / D, scalar2=EPS,
                op0=ALU.mult, op1=ALU.add,
            )
            m2e = lnpool.tile([128, QT], FP32, tag="m2e")
            nc.vector.scalar_tensor_tensor(   # mean^2
                out=m2e, in0=mean, scalar=1.0, in1=mean,
                op0=ALU.mult, op1=ALU.mult,
            )
            nc.vector.tensor_tensor(veps, veps, m2e, op=ALU.subtract)
            # rstd = rsqrt(veps) via bit-trick seed + 2 Newton steps.
            rbits = lnpool.tile([128, QT], I32, tag="rbits")
            nc.vector.tensor_scalar(
                out=rbits, in0=veps.bitcast(I32), scalar1=one_i[:, 0:1],
                scalar2=None, op0=ALU.arith_shift_right,
            )
            nc.vector.tensor_tensor(
                out=rbits, in0=magic_sb.to_broadcast([128, QT]), in1=rbits,
                op=ALU.subtract,
            )
            rstd = rbits.bitcast(FP32)
            for _ in range(1):
                nt1 = lnpool.tile([128, QT], FP32, tag="nt1")
                nc.vector.tensor_mul(nt1, rstd, rstd)
                nc.vector.tensor_mul(nt1, nt1, veps)
                nc.vector.tensor_scalar(
                    out=nt1, in0=nt1, scalar1=-0.5, scalar2=1.5,
                    op0=ALU.mult, op1=ALU.add,
                )
                nc.vector.tensor_mul(rstd, rstd, nt1)
            mean_bf = lnpool.tile([128, QT], BF, tag="mean_bf")
            nc.vector.tensor_copy(mean_bf, mean)
            rstd_bf = lnpool.tile([128, QT], BF, tag="rstd_bf")
            nc.vector.tensor_copy(rstd_bf, rstd)
            yout = ypool.tile([128, MD, QT], BF, tag="yout", bufs=2)
            nc.vector.tensor_tensor(
                yout, y_sb, bcast(mean_bf[:, :], MD), op=ALU.subtract
            )
            nc.vector.tensor_mul(yout, yout, bcast(rstd_bf[:, :], MD))
            if use_gamma_beta:
                for dt in range(MD):
                    nc.vector.tensor_scalar(
                        out=yout[:, dt, :], in0=yout[:, dt, :],
                        scalar1=gam_sb[:, dt:dt + 1],
                        scalar2=bet_sb[:, dt:dt + 1],
                        op0=ALU.mult, op1=ALU.add,
                    )
            nc.sync.dma_start(out=y_r[:, :, g0:g0 + QT], in_=yout)

        d_chains(0)
        for q in range(1, NQ):
            d_chains(q)
            d_finish(q - 1)
        d_finish(NQ - 1)
    nc.compile()
    return nc


_CACHE: dict = {}


def _get_nc(flags):
    if flags not in _CACHE:
        _CACHE[flags] = build_nc(flags)
    return _CACHE[flags]


def kernel(x, w1, b1, w2, b2, wv_w, wv_b, merge_w, merge_b, gamma, beta):
    x = np.asarray(x, dtype=np.float32)
    w1 = np.asarray(w1, dtype=np.float32)
    b1 = np.asarray(b1, dtype=np.float32)
    w2 = np.asarray(w2, dtype=np.float32)
    b2 = np.asarray(b2, dtype=np.float32)
    wv_w = np.asarray(wv_w, dtype=np.float32)
    wv_b = np.asarray(wv_b, dtype=np.float32)
    merge_w = np.asarray(merge_w, dtype=np.float32)
    merge_b = np.asarray(merge_b, dtype=np.float32)
    gamma = np.asarray(gamma, dtype=np.float32)
    beta = np.asarray(beta, dtype=np.float32)

    m2h = 0.5 * merge_w[D:]
    wvm = wv_w @ m2h
    k2 = wv_b @ m2h
    use_gamma_beta = not (np.all(gamma == 1.0) and np.all(beta == 0.0))
    use_merge_b = bool(np.any(merge_b != 0.0))
    use_b1 = bool(np.any(b1 != 0.0))
    use_k2 = bool(np.any(k2 != 0.0))
    b2_half = 0.5 * float(b2[0])
    flags = (use_gamma_beta, use_merge_b, use_b1, use_k2, b2_half)
    nc = _get_nc(flags)

    F8NP = ml_dtypes.float8_e4m3fn
    x2 = x.reshape(B * T, D)
    shared = {
        "w1a": (64.0 * w1[:D]).astype(F8NP),
        "w1c": (64.0 * w1[D:]).astype(F8NP),
        "wvm": wvm.astype(BF16),
        "m1": merge_w[:D].astype(BF16),
        "w2rep": np.ascontiguousarray(
            np.broadcast_to((64.0 * w2).reshape(H, 1), (H, 128))
        ).astype(F8NP),
        "ident": np.eye(128, dtype=np.float32).astype(BF16),
    }
    if use_b1:
        shared["b1r"] = np.ascontiguousarray(64.0 * b1.reshape(MH, 128).T)
    if use_k2:
        shared["k2r"] = np.ascontiguousarray(k2.reshape(MD, 128).T)
    if use_gamma_beta:
        shared["gamr"] = np.ascontiguousarray(gamma.reshape(MD, 128).T)
        shared["betr"] = np.ascontiguousarray(beta.reshape(MD, 128).T)
    if use_merge_b:
        shared["mbr"] = np.ascontiguousarray(merge_b.reshape(MD, 128).T)

    in_maps = []
    for c in range(NCORES):
        t0 = c * NTOK
        xs = np.zeros((GRID, D), np.float32)
        xs[HALO:] = x2[t0:t0 + NTOK]
        if t0 % T != 0:  # halo stays inside the same batch element
            xs[:HALO] = x2[t0 - HALO:t0]
        m = dict(shared)
        xsT = np.ascontiguousarray(xs.T)
        m["xT"] = xsT.astype(BF16)
        m["xT8"] = xsT.astype(F8NP)
        in_maps.append(m)

    res = run_bass_kernel_spmd(nc, in_maps, core_ids=list(range(NCORES)))
    out = np.concatenate([r["y"].T for r in res.results], axis=0)
    return out.reshape(B, T, D).astype(np.float32)
